# revision 1
# baseline (speedup 1.0000x reference)
"""Trainium2 Bass kernel for nn_Conv1dBlock (LIF spikes -> Conv1d(k=5, same) -> GroupNorm).

Contract: kernel(**inputs) takes FULL inputs (x [4,64,256,512] f32, conv_w
[256,256,5], conv_b/gamma/beta [256]) and returns the FULL [4,64,256,512] f32
output. Internally shards data-parallel over B across 8 NeuronCores.

Per-core algorithm (B_loc = 8):
  - LIF (VectorE, fp32, op-order bit-matching the reference):
      d = x - v; v = 0.5*d + v; s = (v >= 0.5) -> bf16; v = (v < 0.5) * v
  - Conv1d as 5 shifted matmuls per (ci_tile, co_tile) accumulated in PSUM.
    Weights split W = hi + lo (both bf16); spikes are exactly representable
    in bf16, so the pair of matmuls reproduces fp32-level accuracy (~2.5e-6).
  - GroupNorm without ever adding conv bias to the [128,512] data:
      r = sum_l y, q = sum_l y^2 (ScalarE activation accum_out)
      t1 = r + 512 b ; t2 = q + 2 b r + 512 b^2   (per-channel, tiny)
      group sums via ones-block matmul; mu/var/rsqrt on 4-8 lanes;
      broadcast back via ones matmul; out = y*A + B on ScalarE where
      A = kappa*gamma, B = (b - mu)*A + beta.
"""

import numpy as np
import ml_dtypes

T, B_FULL, C, L, K = 4, 64, 256, 512, 5
N_CORES = 8
B_LOC = B_FULL // N_CORES
G = 8            # groups
GPC = C // G     # 32 channels per group
CT = 2           # 128-channel tiles
EPS = 1e-5
NORM_N = GPC * L  # 32*512 elements per group

_COMPILED = {}


def _build_program():
    import concourse.bass as bass
    import concourse.tile as tile
    from concourse import bacc, mybir

    f32 = mybir.dt.float32
    bf16 = mybir.dt.bfloat16
    Alu = mybir.AluOpType
    Act = mybir.ActivationFunctionType

    nc = bacc.Bacc(
        "TRN2",
        target_bir_lowering=False,
        debug=False,
        num_devices=N_CORES,
    )

    x_d = nc.dram_tensor("x", [T, B_LOC, C, L], f32, kind="ExternalInput").ap()
    # [ci, prec(hi/lo), k, ci_t, co_t, co]
    w_d = nc.dram_tensor("w", [128, 2, K, 2, CT, 128], bf16, kind="ExternalInput").ap()
    # [co, field, co_t]; fields: b, gamma, beta, 512b, 2b, 512b^2
    chan_d = nc.dram_tensor("chan", [128, 6, CT], f32, kind="ExternalInput").ap()
    onesg_d = nc.dram_tensor("onesg", [128, 4], bf16, kind="ExternalInput").ap()
    onesb_d = nc.dram_tensor("onesb", [128, 128], bf16, kind="ExternalInput").ap()
    y_d = nc.dram_tensor("y", [T, B_LOC, C, L], f32, kind="ExternalOutput").ap()

    with tile.TileContext(nc) as tc:
        with (
            tc.tile_pool(name="singles", bufs=1) as singles,
            tc.tile_pool(name="xp", bufs=12) as xp,
            tc.tile_pool(name="sp", bufs=6) as sp,
            tc.tile_pool(name="dp", bufs=4) as dp,
            tc.tile_pool(name="ysb", bufs=8) as ysb,
            tc.tile_pool(name="smallsb", bufs=6) as smallsb,
            tc.tile_pool(name="ypsum", bufs=6, space="PSUM") as ypsum,
            tc.tile_pool(name="spsum", bufs=2, space="PSUM") as spsum,
        ):
            # first two input tiles before the parameter DMAs so the LIF
            # chain (the startup critical path) begins ASAP
            early_x = {}
            for b in range(2):
                xt = xp.tile([128, 2, L], f32)
                nc.sync.dma_start(
                    out=xt[:], in_=x_d[0, b].rearrange("(i p) l -> p i l", p=128)
                )
                early_x[(0, b)] = xt

            # ---- constants / parameters in SBUF ----
            w_s = singles.tile([128, 2, K, 2, CT, 128], bf16)
            nc.sync.dma_start(out=w_s[:], in_=w_d[:])
            chan = singles.tile([128, 6, CT], f32)
            nc.sync.dma_start(out=chan[:], in_=chan_d[:])
            onesg = singles.tile([128, 4], bf16)
            nc.sync.dma_start(out=onesg[:], in_=onesg_d[:])
            onesb = singles.tile([128, 128], bf16)
            nc.sync.dma_start(out=onesb[:], in_=onesb_d[:])
            eps_t = singles.tile([128, 1], f32)
            nc.vector.memset(eps_t[:], EPS)

            # persistent LIF membrane state per local batch element
            v_tiles = []
            for b in range(B_LOC):
                vt = singles.tile([128, 2, L], f32, tag=f"v{b}")
                nc.gpsimd.memset(vt[:], 0.0)
                v_tiles.append(vt)

            def chan_col(field, ct):
                return chan[:, field, ct : ct + 1]

            # tap -> (rhs_lo, rhs_hi, out_lo, out_hi) column ranges
            tap_slices = []
            for k in range(K):
                d = k - 2
                if d >= 0:
                    tap_slices.append((d, L, 0, L - d))
                else:
                    tap_slices.append((0, L + d, -d, L))

            def emit_tail(pend):
                """Per-channel stat corrections, group-sum + broadcast matmuls,
                A/B, normalize, store for a sample whose conv + ACT stat
                accumulation are already emitted. Deferred one sample so the
                small PE matmuls never stall TensorE and the DVE chain never
                blocks the next sample's LIF. All small matmuls are bf16
                (fp32 matmuls double-issue and disable FWL); fp32 values are
                carried through them as bf16 hi/lo (/lolo) splits."""
                t, b, small_ps, stats_tiles, statsb, y_sbs = pend
                for ct in range(CT):
                    stats = stats_tiles[ct]
                    r = stats[:, 0:1]
                    q = stats[:, 1:2]
                    # t1 = r + 512 b  (bf16 hi + lo)
                    nc.vector.tensor_add(
                        out=statsb[:, ct, 0, 0:1], in0=r, in1=chan_col(3, ct)
                    )
                    nc.vector.scalar_tensor_tensor(
                        out=statsb[:, ct, 0, 1:2], in0=r, scalar=chan_col(3, ct),
                        in1=statsb[:, ct, 0, 0:1], op0=Alu.add, op1=Alu.subtract,
                    )
                    # t2 = (r * 2b + q) + 512 b^2  (f32, then bf16 hi + lo)
                    nc.vector.scalar_tensor_tensor(
                        out=stats[:, 2:3], in0=r, scalar=chan_col(4, ct),
                        in1=q, op0=Alu.mult, op1=Alu.add,
                    )
                    nc.vector.tensor_add(
                        out=stats[:, 3:4], in0=stats[:, 2:3], in1=chan_col(5, ct)
                    )
                    nc.vector.tensor_copy(out=statsb[:, ct, 1, 0:1], in_=stats[:, 3:4])
                    nc.vector.tensor_sub(
                        out=statsb[:, ct, 1, 1:2], in0=stats[:, 3:4],
                        in1=statsb[:, ct, 1, 0:1],
                    )
                # group sums: hi/lo parts summed by PSUM accumulation
                # (two matmuls, both co-tiles at once); out [4, (ct, stat)]
                sbv = statsb.rearrange("p c s h -> p h c s")
                for h in range(2):
                    nc.tensor.matmul(
                        small_ps[0:4, 0:4], onesg[:], sbv[:, h],
                        start=(h == 0), stop=(h == 1),
                    )
                gsum = small_ps[0:4, 0:4].rearrange("p (c s) -> p c s", c=2)
                mk = smallsb.tile([128, 2, 2], f32)  # [grp, ct, (mu,kappa)]
                nc.gpsimd.memset(mk[:], 0.0)
                m2 = smallsb.tile([4, 2], f32)
                vr = smallsb.tile([4, 2], f32)
                mu_v = mk[0:4, :, 0]
                nc.vector.tensor_scalar(
                    out=mu_v, in0=gsum[:, :, 0], scalar1=1.0 / NORM_N,
                    scalar2=None, op0=Alu.mult,
                )
                nc.vector.tensor_mul(out=m2[:], in0=mu_v, in1=mu_v)
                nc.vector.scalar_tensor_tensor(
                    out=vr[:], in0=gsum[:, :, 1], scalar=1.0 / NORM_N, in1=m2[:],
                    op0=Alu.mult, op1=Alu.subtract,
                )
                nc.scalar.activation(
                    out=vr[:], in_=vr[:], func=Act.Sqrt, bias=eps_t[0:4],
                )
                nc.vector.reciprocal(out=mk[0:4, :, 1], in_=vr[:])

                # bf16 3-way split of (mu, kappa) for the broadcast matmul
                mkb = smallsb.tile([128, 2, 2, 3], bf16)
                mr = smallsb.tile([128, 2, 2], f32)
                nc.gpsimd.memset(mkb[:], 0.0)
                nc.vector.tensor_copy(out=mkb[0:4, :, :, 0], in_=mk[0:4])
                nc.vector.tensor_sub(
                    out=mr[0:4], in0=mk[0:4], in1=mkb[0:4, :, :, 0]
                )
                nc.vector.tensor_copy(out=mkb[0:4, :, :, 1], in_=mr[0:4])
                nc.vector.tensor_sub(
                    out=mkb[0:4, :, :, 2], in0=mr[0:4], in1=mkb[0:4, :, :, 1]
                )
                # broadcast: 3 split parts summed by PSUM accumulation;
                # out [128, (ct, muk)]
                mbv = mkb.rearrange("p c s j -> p j c s")
                for j in range(3):
                    nc.tensor.matmul(
                        small_ps[:, 4:8], onesb[:], mbv[:, j],
                        start=(j == 0), stop=(j == 2),
                    )
                bcv = small_ps[:, 4:8].rearrange("p (c s) -> p c s", c=2)
                for ct in range(CT):
                    ab = smallsb.tile([128, 4], f32)
                    # A = kappa * gamma
                    nc.vector.tensor_mul(
                        out=ab[:, 0:1], in0=bcv[:, ct, 1:2], in1=chan_col(1, ct)
                    )
                    # B = (b - mu) * A + beta
                    nc.vector.tensor_sub(
                        out=ab[:, 2:3], in0=chan_col(0, ct), in1=bcv[:, ct, 0:1]
                    )
                    nc.vector.scalar_tensor_tensor(
                        out=ab[:, 1:2], in0=ab[:, 2:3], scalar=ab[:, 0:1],
                        in1=chan_col(2, ct), op0=Alu.mult, op1=Alu.add,
                    )
                    # out = y * A + B  (ScalarE affine, in place on y_sb)
                    y_sb = y_sbs[ct]
                    nc.scalar.activation(
                        out=y_sb[:], in_=y_sb[:], func=Act.Identity,
                        bias=ab[:, 1:2], scale=ab[:, 0:1],
                    )
                    nc.gpsimd.dma_start(
                        out=y_d[t, b].rearrange("(i p) l -> p i l", p=128)[:, ct, :],
                        in_=y_sb[:],
                    )

            pending = None
            for t in range(T):
                for b in range(B_LOC):
                    xt = early_x.pop((t, b), None)
                    if xt is None:
                        xt = xp.tile([128, 2, L], f32)
                        nc.sync.dma_start(
                            out=xt[:],
                            in_=x_d[t, b].rearrange("(i p) l -> p i l", p=128),
                        )
                    v = v_tiles[b]
                    st = sp.tile([128, 2, L], bf16)
                    d_t = dp.tile([128, 2, L], f32)
                    # LIF step (all [128, 2, 512] views)
                    nc.vector.tensor_sub(out=d_t[:], in0=xt[:], in1=v[:])
                    nc.vector.scalar_tensor_tensor(
                        out=v[:], in0=d_t[:], scalar=0.5, in1=v[:],
                        op0=Alu.mult, op1=Alu.add,
                    )
                    nc.vector.tensor_scalar(
                        out=st[:], in0=v[:], scalar1=0.5, scalar2=None,
                        op0=Alu.is_ge,
                    )
                    nc.vector.scalar_tensor_tensor(
                        out=v[:], in0=v[:], scalar=0.5, in1=v[:],
                        op0=Alu.is_lt, op1=Alu.mult,
                    )

                    # conv + stats per co-tile
                    # gsum ct at cols ct*2:(ct+1)*2 ; bcast ct at 4+2ct:6+2ct
                    small_ps = spsum.tile([128, 20], f32)
                    statsb = smallsb.tile([128, 2, 2, 2], bf16)
                    stats_tiles = []
                    y_sbs = []
                    for ct in range(CT):
                        yp = ypsum.tile([128, L], f32)
                        # matmul order: full-width center tap first (start=True)
                        mm_list = []
                        for prec in range(2):
                            for ci_t in range(2):
                                for k in range(K):
                                    mm_list.append((prec, ci_t, k))
                        mm_list.remove((0, 0, 2))
                        mm_list.insert(0, (0, 0, 2))
                        n_mm = len(mm_list)
                        for i, (prec, ci_t, k) in enumerate(mm_list):
                            rl, rh, ol, oh = tap_slices[k]
                            nc.tensor.matmul(
                                yp[:, ol:oh],
                                w_s[:, prec, k, ci_t, ct, :],
                                st[:, ci_t, rl:rh],
                                start=(i == 0),
                                stop=(i == n_mm - 1),
                                skip_group_check=True,
                            )
                        y_sb = ysb.tile([128, L], f32)
                        stats = smallsb.tile([128, 4], f32)
                        # r = sum_l y  (and copy PSUM -> SBUF)
                        nc.scalar.activation(
                            out=y_sb[:], in_=yp[:], func=Act.Copy,
                            accum_out=stats[:, 0:1],
                        )
                        # q = sum_l y^2 (squares PSUM in place; last PSUM use)
                        nc.scalar.activation(
                            out=yp[:], in_=yp[:], func=Act.Square,
                            accum_out=stats[:, 1:2],
                        )
                        stats_tiles.append(stats)
                        y_sbs.append(y_sb)

                    if pending is not None:
                        emit_tail(pending)
                    pending = (t, b, small_ps, stats_tiles, statsb, y_sbs)
            emit_tail(pending)

    nc.compile()
    return nc


def _prep_host_inputs(x, conv_w, conv_b, gamma, beta):
    x = np.asarray(x, dtype=np.float32)
    conv_w = np.asarray(conv_w, dtype=np.float32)
    conv_b = np.asarray(conv_b, dtype=np.float32)
    gamma = np.asarray(gamma, dtype=np.float32)
    beta = np.asarray(beta, dtype=np.float32)

    # lhsT tiles: [ci, prec, k, ci_t, co_t, co]
    Wt = conv_w.transpose(1, 0, 2)                      # [ci_g, co_g, k]
    W6 = Wt.reshape(2, 128, CT, 128, K)                 # [ci_t, ci, co_t, co, k]
    whi32 = W6.astype(ml_dtypes.bfloat16).astype(np.float32)
    wlo = (W6 - whi32).astype(ml_dtypes.bfloat16)
    whi = W6.astype(ml_dtypes.bfloat16)
    w_host = np.stack(
        [whi.transpose(1, 4, 0, 2, 3), wlo.transpose(1, 4, 0, 2, 3)], axis=1
    )                                                   # [ci, prec, k, ci_t, co_t, co]
    w_host = np.ascontiguousarray(w_host)

    b = conv_b
    fields = np.stack(
        [b, gamma, beta, np.float32(L) * b, np.float32(2.0) * b,
         np.float32(L) * b * b]
    )                                                   # [6, 256]
    chan = np.ascontiguousarray(fields.reshape(6, CT, 128).transpose(2, 0, 1))

    onesg = np.zeros((128, 4), ml_dtypes.bfloat16)
    for ci in range(128):
        onesg[ci, ci // GPC] = 1.0
    onesb = np.zeros((128, 128), ml_dtypes.bfloat16)
    for co in range(128):
        onesb[co // GPC, co] = 1.0

    shards = []
    for i in range(N_CORES):
        shards.append(
            {
                "x": np.ascontiguousarray(x[:, i * B_LOC : (i + 1) * B_LOC]),
                "w": w_host,
                "chan": chan,
                "onesg": onesg,
                "onesb": onesb,
            }
        )
    return shards


def kernel(x, conv_w, conv_b, gamma, beta, _trace=False):
    from concourse.bass_utils import run_bass_kernel_spmd

    if "nc" not in _COMPILED:
        _COMPILED["nc"] = _build_program()
    nc = _COMPILED["nc"]

    in_maps = _prep_host_inputs(x, conv_w, conv_b, gamma, beta)
    res = run_bass_kernel_spmd(
        nc, in_maps, list(range(N_CORES)), trace=_trace
    )
    out = np.concatenate([r["y"] for r in res.results], axis=1)
    _COMPILED["last_result"] = res
    return out



# revision 8
# speedup vs baseline: 1.7487x; 1.7487x over previous
"""Trainium2 Bass kernel for nn_Conv1dBlock (LIF spikes -> Conv1d(k=5, same) -> GroupNorm).

Contract: kernel(**inputs) takes FULL inputs (x [4,64,256,512] f32, conv_w
[256,256,5], conv_b/gamma/beta [256]) and returns the FULL [4,64,256,512] f32
output. Internally shards data-parallel over B across 8 NeuronCores.

Per-core algorithm (B_loc = 8), tuned for rel-err budget 2e-2:
  - LIF bit-matches the reference op order (d = x - v; v += 0.5 d;
    s = v >= 0.5; v *= (v < 0.5)).  The d subtract runs on GpSimd; the
    t=0 step collapses to v = 0.5 x and t=3 skips the reset.
  - Conv1d as 5 shifted matmuls per (ci_tile, co_tile) accumulated in PSUM
    with SINGLE-precision bf16 weights (measured end-to-end err ~1.7e-3).
  - GroupNorm stats: ScalarE Copy/Square passes with accum_out give
    r = sum_l y and q = sum_l y^2 per channel (conv bias never added to the
    big tensor; stats corrected per channel: t1 = r + 512 b,
    t2 = q + 2 b r + 512 b^2).  Group sums + broadcast via tiny bf16
    matmuls.  All tail arithmetic is batched over quads of 4 samples so the
    per-op DVE overhead amortizes 4x.
  - Normalize out = y*A + B runs on GpSimd tensor_scalar with per-partition
    AP scalars; output DMA dispatch on the sync queue.
"""

import numpy as np
import ml_dtypes

T, B_FULL, C, L, K = 4, 64, 256, 512, 5
N_CORES = 8
B_LOC = B_FULL // N_CORES
G = 8            # groups
GPC = C // G     # 32 channels per group
CT = 2           # 128-channel tiles
EPS = 1e-5
NORM_N = GPC * L  # 32*512 elements per group
QUAD = 4          # samples per batched groupnorm tail

_COMPILED = {}


def _build_program():
    import concourse.bass as bass
    import concourse.tile as tile
    from concourse import bacc, mybir

    f32 = mybir.dt.float32
    bf16 = mybir.dt.bfloat16
    Alu = mybir.AluOpType
    Act = mybir.ActivationFunctionType

    nc = bacc.Bacc(
        "TRN2",
        target_bir_lowering=False,
        debug=False,
        num_devices=N_CORES,
    )

    x_d = nc.dram_tensor("x", [T, B_LOC, C, L], f32, kind="ExternalInput").ap()
    # [ci, k, ci_t, co_t, co] single-precision bf16 weights
    w_d = nc.dram_tensor("w", [128, K, 2, CT, 128], bf16, kind="ExternalInput").ap()
    # [co, field, smp, ct]; fields: 512b, 2b, 512b^2, gamma, b, beta
    # (duplicated over the 4 quad sample slots)
    consts_d = nc.dram_tensor("consts", [128, 6, QUAD, CT], f32, kind="ExternalInput").ap()
    ind4_d = nc.dram_tensor("ind4", [128, 4], bf16, kind="ExternalInput").ap()
    onesb4_d = nc.dram_tensor("onesb4", [4, 128], bf16, kind="ExternalInput").ap()
    y_d = nc.dram_tensor("y", [T, B_LOC, C, L], f32, kind="ExternalOutput").ap()

    with tile.TileContext(nc) as tc:
        with (
            tc.tile_pool(name="singles", bufs=1) as singles,
            tc.tile_pool(name="xp", bufs=8) as xp,
            tc.tile_pool(name="sp", bufs=6) as sp,
            tc.tile_pool(name="ysb", bufs=20) as ysb,
            tc.tile_pool(name="smallsb", bufs=3) as smallsb,
            tc.tile_pool(name="ypsum", bufs=6, space="PSUM") as ypsum,
            tc.tile_pool(name="spsum", bufs=1, space="PSUM") as spsum,
        ):
            # first two input tiles before the parameter DMAs so the LIF
            # chain (the startup critical path) begins ASAP
            early_x = {}
            for b in range(2):
                xt = xp.tile([128, 2, L], f32)
                nc.sync.dma_start(
                    out=xt[:], in_=x_d[0, b].rearrange("(i p) l -> p i l", p=128)
                )
                early_x[(0, b)] = xt

            # ---- constants / parameters in SBUF ----
            w_s = singles.tile([128, K, 2, CT, 128], bf16)
            nc.sync.dma_start(out=w_s[:], in_=w_d[:])
            consts = singles.tile([128, 6, QUAD, CT], f32)
            nc.sync.dma_start(out=consts[:], in_=consts_d[:])
            ind4 = singles.tile([128, 4], bf16)
            nc.sync.dma_start(out=ind4[:], in_=ind4_d[:])
            onesb4 = singles.tile([4, 128], bf16)
            nc.sync.dma_start(out=onesb4[:], in_=onesb4_d[:])
            eps_t = singles.tile([128, 1], f32)
            nc.vector.memset(eps_t[:], EPS)

            # persistent LIF membrane state per local batch element; no
            # memset needed: the t=0 step overwrites v entirely.
            v_tiles = []
            for b in range(B_LOC):
                vt = singles.tile([128, 2, L], f32, tag=f"v{b}")
                v_tiles.append(vt)

            # tap -> (rhs_lo, rhs_hi, out_lo, out_hi) column ranges
            tap_slices = []
            for k in range(K):
                d = k - 2
                if d >= 0:
                    tap_slices.append((d, L, 0, L - d))
                else:
                    tap_slices.append((0, L + d, -d, L))
            mm_list = [(ci_t, k) for ci_t in range(2) for k in range(K)]
            mm_list.remove((0, 2))
            mm_list.insert(0, (0, 2))  # full-width center tap first (start=True)

            def emit_tail(quad, rq, ysbs):
                """Batched groupnorm tail for a quad of 4 samples: per-channel
                bias corrections, group-sum + broadcast matmuls (bf16), A/B
                affine coefficients, then per-sample normalize + store."""
                # t1 = r + 512 b ; t2 = (r * 2b + q) + 512 b^2  -> bf16
                t12 = smallsb.tile([128, 2, QUAD, CT], bf16)
                nc.vector.tensor_add(out=t12[:, 0], in0=rq[:, 0], in1=consts[:, 0])
                tmp = smallsb.tile([128, QUAD, CT], f32)
                nc.vector.tensor_mul(out=tmp[:], in0=rq[:, 0], in1=consts[:, 1])
                nc.vector.tensor_add(out=tmp[:], in0=tmp[:], in1=rq[:, 1])
                nc.vector.tensor_add(out=t12[:, 1], in0=tmp[:], in1=consts[:, 2])
                # group sums over the 32-channel blocks: [4, (stat, smp, ct)]
                gs = spsum.tile([4, 2, QUAD, CT], f32)
                nc.tensor.matmul(gs[:], ind4[:], t12[:], start=True, stop=True)
                # mu = T1/N ; varN = T2 - T1*mu ; kappa = rsqrt(varN/N + eps)
                mu = smallsb.tile([4, QUAD, CT], f32)
                nc.vector.tensor_scalar(
                    out=mu[:], in0=gs[0:4, 0], scalar1=1.0 / NORM_N,
                    scalar2=None, op0=Alu.mult,
                )
                vn = smallsb.tile([4, QUAD, CT], f32)
                nc.vector.tensor_mul(out=vn[:], in0=gs[0:4, 0], in1=mu[:])
                nc.vector.tensor_sub(out=vn[:], in0=gs[0:4, 1], in1=vn[:])
                mk = smallsb.tile([4, 2, QUAD, CT], bf16)
                nc.vector.tensor_copy(out=mk[:, 0], in_=mu[:])
                nc.scalar.activation(
                    out=vn[:], in_=vn[:], func=Act.Sqrt,
                    bias=eps_t[0:4], scale=1.0 / NORM_N,
                )
                with nc.allow_low_precision(reason="kappa in bf16 is fine for 2e-2 budget"):
                    nc.vector.reciprocal(out=mk[:, 1], in_=vn[:])
                # broadcast mu/kappa back to 128 channels: [128, (muk, smp, ct)]
                bc = spsum.tile([128, 2, QUAD, CT], f32)
                nc.tensor.matmul(bc[:], onesb4[:], mk[:], start=True, stop=True)
                # A = kappa*gamma ; B = (b - mu)*A + beta
                ab = smallsb.tile([128, 2, QUAD, CT], f32)
                nc.vector.tensor_mul(out=ab[:, 0], in0=bc[:, 1], in1=consts[:, 3])
                scr = smallsb.tile([128, QUAD, CT], f32)
                nc.vector.tensor_sub(out=scr[:], in0=consts[:, 4], in1=bc[:, 0])
                nc.vector.tensor_mul(out=scr[:], in0=scr[:], in1=ab[:, 0])
                nc.vector.tensor_add(out=ab[:, 1], in0=scr[:], in1=consts[:, 5])
                for si, (t, b) in enumerate(quad):
                    for ct in range(CT):
                        y_sb = ysbs[si][ct]
                        # out = y*A + B on ScalarE (own SBUF port; GpSimd
                        # shares ports with DVE so it must stay quiet)
                        nc.scalar.activation(
                            out=y_sb[:], in_=y_sb[:], func=Act.Identity,
                            bias=ab[:, 1, si, ct : ct + 1],
                            scale=ab[:, 0, si, ct : ct + 1],
                        )
                        nc.gpsimd.dma_start(
                            out=y_d[t, b].rearrange("(i p) l -> p i l", p=128)[:, ct, :],
                            in_=y_sb[:],
                        )

            samples = [(t, b) for t in range(T) for b in range(B_LOC)]
            pending = None
            for q0 in range(0, len(samples), QUAD):
                quad = samples[q0 : q0 + QUAD]
                rq = smallsb.tile([128, 2, QUAD, CT], f32)  # (stat, smp, ct)
                ysbs = []
                for si, (t, b) in enumerate(quad):
                    xt = early_x.pop((t, b), None)
                    if xt is None:
                        xt = xp.tile([128, 2, L], f32)
                        nc.sync.dma_start(
                            out=xt[:],
                            in_=x_d[t, b].rearrange("(i p) l -> p i l", p=128),
                        )
                    v = v_tiles[b]
                    # LIF step, bit-matching the reference op order
                    if t == 0:
                        nc.vector.tensor_scalar(
                            out=v[:], in0=xt[:], scalar1=0.5, scalar2=None,
                            op0=Alu.mult,
                        )
                    else:
                        nc.vector.tensor_sub(out=xt[:], in0=xt[:], in1=v[:])
                        nc.vector.scalar_tensor_tensor(
                            out=v[:], in0=xt[:], scalar=0.5, in1=v[:],
                            op0=Alu.mult, op1=Alu.add,
                        )
                    st = sp.tile([128, 2, L], bf16)
                    nc.vector.tensor_scalar(
                        out=st[:], in0=v[:], scalar1=0.5, scalar2=None,
                        op0=Alu.is_ge,
                    )
                    if t < T - 1:
                        nc.vector.scalar_tensor_tensor(
                            out=v[:], in0=v[:], scalar=0.5, in1=v[:],
                            op0=Alu.is_lt, op1=Alu.mult,
                        )

                    # conv + stats per co-tile
                    pair = []
                    for ct in range(CT):
                        yp = ypsum.tile([128, L], f32)
                        for i, (ci_t, k) in enumerate(mm_list):
                            rl, rh, ol, oh = tap_slices[k]
                            nc.tensor.matmul(
                                yp[:, ol:oh],
                                w_s[:, k, ci_t, ct, :],
                                st[:, ci_t, rl:rh],
                                start=(i == 0),
                                stop=(i == len(mm_list) - 1),
                                skip_group_check=True,
                            )
                        y_sb = ysb.tile([128, L], f32)
                        # r = sum_l y (and copy PSUM -> SBUF)
                        nc.scalar.activation(
                            out=y_sb[:], in_=yp[:], func=Act.Copy,
                            accum_out=rq[:, 0, si, ct : ct + 1],
                        )
                        # q = sum_l y^2 (squares PSUM in place; last PSUM use)
                        nc.scalar.activation(
                            out=yp[:], in_=yp[:], func=Act.Square,
                            accum_out=rq[:, 1, si, ct : ct + 1],
                        )
                        pair.append(y_sb)
                    ysbs.append(pair)

                if pending is not None:
                    emit_tail(*pending)
                pending = (quad, rq, ysbs)
            emit_tail(*pending)

    nc.compile()
    return nc


def _prep_host_inputs(x, conv_w, conv_b, gamma, beta):
    x = np.asarray(x, dtype=np.float32)
    conv_w = np.asarray(conv_w, dtype=np.float32)
    conv_b = np.asarray(conv_b, dtype=np.float32)
    gamma = np.asarray(gamma, dtype=np.float32)
    beta = np.asarray(beta, dtype=np.float32)

    # lhsT tiles: [ci, k, ci_t, co_t, co], single-precision bf16
    Wt = conv_w.transpose(1, 0, 2)                      # [ci_g, co_g, k]
    W6 = Wt.reshape(2, 128, CT, 128, K)                 # [ci_t, ci, co_t, co, k]
    w_host = np.ascontiguousarray(
        W6.astype(ml_dtypes.bfloat16).transpose(1, 4, 0, 2, 3)
    )

    b = conv_b
    fields = np.stack(
        [np.float32(L) * b, np.float32(2.0) * b, np.float32(L) * b * b,
         gamma, b, beta]
    )                                                   # [6, 256]
    f6 = fields.reshape(6, CT, 128)                     # [field, ct, co]
    consts = np.zeros((128, 6, QUAD, CT), np.float32)
    for ct in range(CT):
        consts[:, :, :, ct] = f6[:, ct, :].T[:, :, None]

    ind4 = np.zeros((128, 4), ml_dtypes.bfloat16)
    for ci in range(128):
        ind4[ci, ci // GPC] = 1.0
    onesb4 = np.zeros((4, 128), ml_dtypes.bfloat16)
    for co in range(128):
        onesb4[co // GPC, co] = 1.0

    shards = []
    for i in range(N_CORES):
        shards.append(
            {
                "x": np.ascontiguousarray(x[:, i * B_LOC : (i + 1) * B_LOC]),
                "w": w_host,
                "consts": consts,
                "ind4": ind4,
                "onesb4": onesb4,
            }
        )
    return shards


def kernel(x, conv_w, conv_b, gamma, beta, _trace=False):
    from concourse.bass_utils import run_bass_kernel_spmd

    if "nc" not in _COMPILED:
        _COMPILED["nc"] = _build_program()
    nc = _COMPILED["nc"]

    in_maps = _prep_host_inputs(x, conv_w, conv_b, gamma, beta)
    res = run_bass_kernel_spmd(
        nc, in_maps, list(range(N_CORES)), trace=_trace
    )
    out = np.concatenate([r["y"] for r in res.results], axis=1)
    _COMPILED["last_result"] = res
    return out


# revision 13
# speedup vs baseline: 1.9068x; 1.0904x over previous
"""Trainium2 Bass kernel for nn_Conv1dBlock (LIF spikes -> Conv1d(k=5, same) -> GroupNorm).

Contract: kernel(**inputs) takes FULL inputs (x [4,64,256,512] f32, conv_w
[256,256,5], conv_b/gamma/beta [256]) and returns the FULL [4,64,256,512] f32
output. Internally shards data-parallel over B across 8 NeuronCores.

Per-core algorithm (B_loc = 8), tuned for rel-err budget 2e-2:
  - LIF bit-matches the reference op order (d = x - v; v += 0.5 d;
    s = v >= 0.5; v *= (v < 0.5)).  The d subtract runs on GpSimd; the
    t=0 step collapses to v = 0.5 x and t=3 skips the reset.
  - Conv1d as 5 shifted matmuls per (ci_tile, co_tile) accumulated in PSUM
    with SINGLE-precision bf16 weights (measured end-to-end err ~1.7e-3).
  - GroupNorm stats: ScalarE Copy/Square passes with accum_out give
    r = sum_l y and q = sum_l y^2 per channel (conv bias never added to the
    big tensor; stats corrected per channel: t1 = r + 512 b,
    t2 = q + 2 b r + 512 b^2).  Group sums + broadcast via tiny bf16
    matmuls.  All tail arithmetic is batched over quads of 4 samples so the
    per-op DVE overhead amortizes 4x.
  - Normalize out = y*A + B runs on GpSimd tensor_scalar with per-partition
    AP scalars; output DMA dispatch on the sync queue.
"""

import numpy as np
import ml_dtypes

T, B_FULL, C, L, K = 4, 64, 256, 512, 5
N_CORES = 8
B_LOC = B_FULL // N_CORES
G = 8            # groups
GPC = C // G     # 32 channels per group
CT = 2           # 128-channel tiles
EPS = 1e-5
NORM_N = GPC * L  # 32*512 elements per group
QUAD = 4          # samples per batched groupnorm tail

_COMPILED = {}


def _build_program():
    import concourse.bass as bass
    import concourse.tile as tile
    from concourse import bacc, mybir

    f32 = mybir.dt.float32
    bf16 = mybir.dt.bfloat16
    Alu = mybir.AluOpType
    Act = mybir.ActivationFunctionType

    nc = bacc.Bacc(
        "TRN2",
        target_bir_lowering=False,
        debug=False,
        num_devices=N_CORES,
    )

    x_d = nc.dram_tensor("x", [T, B_LOC, C, L], f32, kind="ExternalInput").ap()
    # [ci, k, ci_t, co_t, co] single-precision bf16 weights
    w_d = nc.dram_tensor("w", [128, K, 2, CT, 128], bf16, kind="ExternalInput").ap()
    # [co, field, smp, ct]; fields: 512b, 2b, 512b^2, gamma, b, beta
    # (duplicated over the 4 quad sample slots)
    consts_d = nc.dram_tensor("consts", [128, 6, QUAD, CT], f32, kind="ExternalInput").ap()
    ind4_d = nc.dram_tensor("ind4", [128, 4], bf16, kind="ExternalInput").ap()
    onesb4_d = nc.dram_tensor("onesb4", [4, 128], bf16, kind="ExternalInput").ap()
    y_d = nc.dram_tensor("y", [T, B_LOC, C, L], f32, kind="ExternalOutput").ap()

    with tile.TileContext(nc) as tc:
        with (
            tc.tile_pool(name="singles", bufs=1) as singles,
            tc.tile_pool(name="xp", bufs=8) as xp,
            tc.tile_pool(name="sp", bufs=6) as sp,
            tc.tile_pool(name="ysb", bufs=20) as ysb,
            tc.tile_pool(name="smallsb", bufs=3) as smallsb,
            tc.tile_pool(name="ypsum", bufs=6, space="PSUM") as ypsum,
            tc.tile_pool(name="spsum", bufs=1, space="PSUM") as spsum,
        ):
            # startup ordering: x(0,0) first (gates the whole LIF chain),
            # then just the center-tap weights (gate the first matmul),
            # then the rest interleaved
            early_x = {}
            xt = xp.tile([128, 2, L], f32, tag="x_e0")
            nc.sync.dma_start(
                out=xt[:], in_=x_d[0, 0].rearrange("(i p) l -> p i l", p=128)
            )
            early_x[(0, 0)] = xt

            w_s = singles.tile([128, K, 2, CT, 128], bf16)
            nc.sync.dma_start(out=w_s[:, 2], in_=w_d[:, 2])
            xt = xp.tile([128, 2, L], f32, tag="x_e1")
            nc.sync.dma_start(
                out=xt[:], in_=x_d[0, 1].rearrange("(i p) l -> p i l", p=128)
            )
            early_x[(0, 1)] = xt
            nc.sync.dma_start(out=w_s[:, 0:2], in_=w_d[:, 0:2])
            nc.sync.dma_start(out=w_s[:, 3:5], in_=w_d[:, 3:5])
            consts = singles.tile([128, 6, QUAD, CT], f32)
            nc.sync.dma_start(out=consts[:], in_=consts_d[:])
            ind4 = singles.tile([128, 4], bf16)
            nc.sync.dma_start(out=ind4[:], in_=ind4_d[:])
            onesb4 = singles.tile([4, 128], bf16)
            nc.sync.dma_start(out=onesb4[:], in_=onesb4_d[:])
            eps_t = singles.tile([128, 1], f32)
            nc.vector.memset(eps_t[:], EPS)

            # persistent LIF membrane state per local batch element; no
            # memset needed: the t=0 step overwrites v entirely.
            v_tiles = []
            for b in range(B_LOC):
                vt = singles.tile([128, 2, L], f32, tag=f"v{b}")
                v_tiles.append(vt)

            # tap -> (rhs_lo, rhs_hi, out_lo, out_hi) column ranges
            tap_slices = []
            for k in range(K):
                d = k - 2
                if d >= 0:
                    tap_slices.append((d, L, 0, L - d))
                else:
                    tap_slices.append((0, L + d, -d, L))
            mm_list = [(ci_t, k) for ci_t in range(2) for k in range(K)]
            mm_list.remove((0, 2))
            mm_list.insert(0, (0, 2))  # full-width center tap first (start=True)

            def emit_tail(quad, rq, ysbs):
                """Batched groupnorm tail for a quad of 4 samples: per-channel
                bias corrections, group-sum + broadcast matmuls (bf16), A/B
                affine coefficients, then per-sample normalize + store."""
                # t1 = r + 512 b ; t2 = (r * 2b + q) + 512 b^2  -> bf16
                t12 = smallsb.tile([128, 2, QUAD, CT], bf16)
                nc.vector.tensor_add(out=t12[:, 0], in0=rq[:, 0], in1=consts[:, 0])
                tmp = smallsb.tile([128, QUAD, CT], f32)
                nc.vector.tensor_mul(out=tmp[:], in0=rq[:, 0], in1=consts[:, 1])
                nc.vector.tensor_add(out=tmp[:], in0=tmp[:], in1=rq[:, 1])
                nc.vector.tensor_add(out=t12[:, 1], in0=tmp[:], in1=consts[:, 2])
                # group sums over the 32-channel blocks: [4, (stat, smp, ct)]
                gs = spsum.tile([4, 2, QUAD, CT], f32)
                nc.tensor.matmul(gs[:], ind4[:], t12[:], start=True, stop=True)
                # mu = T1/N ; varN = T2 - T1*mu ; kappa = rsqrt(varN/N + eps)
                mu = smallsb.tile([4, QUAD, CT], f32)
                nc.vector.tensor_scalar(
                    out=mu[:], in0=gs[0:4, 0], scalar1=1.0 / NORM_N,
                    scalar2=None, op0=Alu.mult,
                )
                vn = smallsb.tile([4, QUAD, CT], f32)
                nc.vector.tensor_mul(out=vn[:], in0=gs[0:4, 0], in1=mu[:])
                nc.vector.tensor_sub(out=vn[:], in0=gs[0:4, 1], in1=vn[:])
                mk = smallsb.tile([4, 2, QUAD, CT], bf16)
                nc.vector.tensor_copy(out=mk[:, 0], in_=mu[:])
                nc.scalar.activation(
                    out=vn[:], in_=vn[:], func=Act.Sqrt,
                    bias=eps_t[0:4], scale=1.0 / NORM_N,
                )
                with nc.allow_low_precision(reason="kappa in bf16 is fine for 2e-2 budget"):
                    nc.vector.reciprocal(out=mk[:, 1], in_=vn[:])
                # broadcast mu/kappa back to 128 channels: [128, (muk, smp, ct)]
                bc = spsum.tile([128, 2, QUAD, CT], f32)
                nc.tensor.matmul(bc[:], onesb4[:], mk[:], start=True, stop=True)
                # A = kappa*gamma ; B = (b - mu)*A + beta
                ab = smallsb.tile([128, 2, QUAD, CT], f32)
                nc.vector.tensor_mul(out=ab[:, 0], in0=bc[:, 1], in1=consts[:, 3])
                scr = smallsb.tile([128, QUAD, CT], f32)
                nc.vector.tensor_sub(out=scr[:], in0=consts[:, 4], in1=bc[:, 0])
                nc.vector.tensor_mul(out=scr[:], in0=scr[:], in1=ab[:, 0])
                nc.vector.tensor_add(out=ab[:, 1], in0=scr[:], in1=consts[:, 5])
                for si, (t, b) in enumerate(quad):
                    for ct in range(CT):
                        y_sb = ysbs[si][ct]
                        # out = y*A + B: split between DVE (tensor_scalar,
                        # 2x mode) and ScalarE so neither becomes critical
                        if ct == 0:
                            nc.vector.tensor_scalar(
                                out=y_sb[:], in0=y_sb[:],
                                scalar1=ab[:, 0, si, ct : ct + 1],
                                scalar2=ab[:, 1, si, ct : ct + 1],
                                op0=Alu.mult, op1=Alu.add,
                            )
                        else:
                            nc.scalar.activation(
                                out=y_sb[:], in_=y_sb[:], func=Act.Identity,
                                bias=ab[:, 1, si, ct : ct + 1],
                                scale=ab[:, 0, si, ct : ct + 1],
                            )
                        dma_q = nc.sync if ct == 0 else nc.gpsimd
                        dma_q.dma_start(
                            out=y_d[t, b].rearrange("(i p) l -> p i l", p=128)[:, ct, :],
                            in_=y_sb[:],
                        )

            samples = [(t, b) for t in range(T) for b in range(B_LOC)]
            quad_state = {}  # quad index -> (quad, rq, ysbs)
            for i, (t, b) in enumerate(samples):
                qi, si = divmod(i, QUAD)
                if si == 0:
                    rq = smallsb.tile([128, 2, QUAD, CT], f32)  # (stat, smp, ct)
                    quad_state[qi] = ([], rq, [])
                quad, rq, ysbs = quad_state[qi]
                quad.append((t, b))
                if True:
                    xt = early_x.pop((t, b), None)
                    if xt is None:
                        xt = xp.tile([128, 2, L], f32)
                        nc.sync.dma_start(
                            out=xt[:],
                            in_=x_d[t, b].rearrange("(i p) l -> p i l", p=128),
                        )
                    v = v_tiles[b]
                    # LIF step, bit-matching the reference op order
                    if t == 0:
                        nc.vector.tensor_scalar(
                            out=v[:], in0=xt[:], scalar1=0.5, scalar2=None,
                            op0=Alu.mult,
                        )
                    else:
                        nc.vector.tensor_sub(out=xt[:], in0=xt[:], in1=v[:])
                        nc.vector.scalar_tensor_tensor(
                            out=v[:], in0=xt[:], scalar=0.5, in1=v[:],
                            op0=Alu.mult, op1=Alu.add,
                        )
                    st = sp.tile([128, 2, L], bf16)
                    nc.vector.tensor_scalar(
                        out=st[:], in0=v[:], scalar1=0.5, scalar2=None,
                        op0=Alu.is_ge,
                    )
                    if t < T - 1:
                        nc.vector.scalar_tensor_tensor(
                            out=v[:], in0=v[:], scalar=0.5, in1=v[:],
                            op0=Alu.is_lt, op1=Alu.mult,
                        )

                    # conv + stats per co-tile
                    pair = []
                    for ct in range(CT):
                        yp = ypsum.tile([128, L], f32)
                        for mi, (ci_t, k) in enumerate(mm_list):
                            rl, rh, ol, oh = tap_slices[k]
                            nc.tensor.matmul(
                                yp[:, ol:oh],
                                w_s[:, k, ci_t, ct, :],
                                st[:, ci_t, rl:rh],
                                start=(mi == 0),
                                stop=(mi == len(mm_list) - 1),
                                skip_group_check=True,
                            )
                        y_sb = ysb.tile([128, L], f32)
                        # r = sum_l y (and copy PSUM -> SBUF)
                        nc.scalar.activation(
                            out=y_sb[:], in_=yp[:], func=Act.Copy,
                            accum_out=rq[:, 0, si, ct : ct + 1],
                        )
                        # q = sum_l y^2 (squares PSUM in place; last PSUM use)
                        nc.scalar.activation(
                            out=yp[:], in_=yp[:], func=Act.Square,
                            accum_out=rq[:, 1, si, ct : ct + 1],
                        )
                        pair.append(y_sb)
                    ysbs.append(pair)

                # emit the previous quad's tail once 2 samples of the current
                # quad are in the queues: late enough that its stats are done
                # by the time DVE reaches it (no head-of-line stall), early
                # enough that only the final quad's tail lands in the drain
                if si == 1 and qi >= 1:
                    emit_tail(*quad_state.pop(qi - 1))
            emit_tail(*quad_state.pop(len(samples) // QUAD - 1))

    nc.compile()
    return nc


def _prep_host_inputs(x, conv_w, conv_b, gamma, beta):
    x = np.asarray(x, dtype=np.float32)
    conv_w = np.asarray(conv_w, dtype=np.float32)
    conv_b = np.asarray(conv_b, dtype=np.float32)
    gamma = np.asarray(gamma, dtype=np.float32)
    beta = np.asarray(beta, dtype=np.float32)

    # lhsT tiles: [ci, k, ci_t, co_t, co], single-precision bf16
    Wt = conv_w.transpose(1, 0, 2)                      # [ci_g, co_g, k]
    W6 = Wt.reshape(2, 128, CT, 128, K)                 # [ci_t, ci, co_t, co, k]
    w_host = np.ascontiguousarray(
        W6.astype(ml_dtypes.bfloat16).transpose(1, 4, 0, 2, 3)
    )

    b = conv_b
    fields = np.stack(
        [np.float32(L) * b, np.float32(2.0) * b, np.float32(L) * b * b,
         gamma, b, beta]
    )                                                   # [6, 256]
    f6 = fields.reshape(6, CT, 128)                     # [field, ct, co]
    consts = np.zeros((128, 6, QUAD, CT), np.float32)
    for ct in range(CT):
        consts[:, :, :, ct] = f6[:, ct, :].T[:, :, None]

    ind4 = np.zeros((128, 4), ml_dtypes.bfloat16)
    for ci in range(128):
        ind4[ci, ci // GPC] = 1.0
    onesb4 = np.zeros((4, 128), ml_dtypes.bfloat16)
    for co in range(128):
        onesb4[co // GPC, co] = 1.0

    shards = []
    for i in range(N_CORES):
        shards.append(
            {
                "x": np.ascontiguousarray(x[:, i * B_LOC : (i + 1) * B_LOC]),
                "w": w_host,
                "consts": consts,
                "ind4": ind4,
                "onesb4": onesb4,
            }
        )
    return shards


def kernel(x, conv_w, conv_b, gamma, beta, _trace=False):
    from concourse.bass_utils import run_bass_kernel_spmd

    if "nc" not in _COMPILED:
        _COMPILED["nc"] = _build_program()
    nc = _COMPILED["nc"]

    in_maps = _prep_host_inputs(x, conv_w, conv_b, gamma, beta)
    res = run_bass_kernel_spmd(
        nc, in_maps, list(range(N_CORES)), trace=_trace
    )
    out = np.concatenate([r["y"] for r in res.results], axis=1)
    _COMPILED["last_result"] = res
    return out


# revision 19
# speedup vs baseline: 1.9539x; 1.0247x over previous
"""Trainium2 Bass kernel for nn_Conv1dBlock (LIF spikes -> Conv1d(k=5, same) -> GroupNorm).

Contract: kernel(**inputs) takes FULL inputs (x [4,64,256,512] f32, conv_w
[256,256,5], conv_b/gamma/beta [256]) and returns the FULL [4,64,256,512] f32
output. Internally shards data-parallel over B across 8 NeuronCores.

Per-core algorithm (B_loc = 8), tuned for rel-err budget 2e-2:
  - LIF bit-matches the reference op order (d = x - v; v += 0.5 d;
    s = v >= 0.5; v *= (v < 0.5)).  The d subtract runs on GpSimd; the
    t=0 step collapses to v = 0.5 x and t=3 skips the reset.
  - Conv1d as 5 shifted matmuls per (ci_tile, co_tile) accumulated in PSUM
    with SINGLE-precision bf16 weights (measured end-to-end err ~1.7e-3).
  - GroupNorm stats: ScalarE Copy/Square passes with accum_out give
    r = sum_l y and q = sum_l y^2 per channel (conv bias never added to the
    big tensor; stats corrected per channel: t1 = r + 512 b,
    t2 = q + 2 b r + 512 b^2).  Group sums + broadcast via tiny bf16
    matmuls.  All tail arithmetic is batched over quads of 4 samples so the
    per-op DVE overhead amortizes 4x.
  - Normalize out = y*A + B runs on GpSimd tensor_scalar with per-partition
    AP scalars; output DMA dispatch on the sync queue.
"""

import numpy as np
import ml_dtypes

T, B_FULL, C, L, K = 4, 64, 256, 512, 5
N_CORES = 8
B_LOC = B_FULL // N_CORES
G = 8            # groups
GPC = C // G     # 32 channels per group
CT = 2           # 128-channel tiles
EPS = 1e-5
NORM_N = GPC * L  # 32*512 elements per group
QUAD = 4          # samples per batched groupnorm tail

_COMPILED = {}


def _build_program():
    import concourse.bass as bass
    import concourse.tile as tile
    from concourse import bacc, mybir

    f32 = mybir.dt.float32
    bf16 = mybir.dt.bfloat16
    Alu = mybir.AluOpType
    Act = mybir.ActivationFunctionType

    nc = bacc.Bacc(
        "TRN2",
        target_bir_lowering=False,
        debug=False,
        num_devices=N_CORES,
    )

    x_d = nc.dram_tensor("x", [T, B_LOC, C, L], f32, kind="ExternalInput").ap()
    # [ci, k, ci_t, co_t, co] single-precision bf16 weights
    w_d = nc.dram_tensor("w", [128, K, 2, CT, 128], bf16, kind="ExternalInput").ap()
    # [co, field, smp, ct]; fields: 512b, 2b, 512b^2, gamma, b, beta
    # (duplicated over the 4 quad sample slots)
    consts_d = nc.dram_tensor("consts", [128, 6, QUAD, CT], f32, kind="ExternalInput").ap()
    ind4_d = nc.dram_tensor("ind4", [128, 4], bf16, kind="ExternalInput").ap()
    onesb4_d = nc.dram_tensor("onesb4", [4, 128], bf16, kind="ExternalInput").ap()
    y_d = nc.dram_tensor("y", [T, B_LOC, C, L], f32, kind="ExternalOutput").ap()

    with tile.TileContext(nc) as tc:
        with (
            tc.tile_pool(name="singles", bufs=1) as singles,
            tc.tile_pool(name="xp", bufs=8) as xp,
            tc.tile_pool(name="sp", bufs=6) as sp,
            tc.tile_pool(name="ysb", bufs=20) as ysb,
            tc.tile_pool(name="smallsb", bufs=3) as smallsb,
            tc.tile_pool(name="ypsum", bufs=5, space="PSUM") as ypsum,
            tc.tile_pool(name="spsum", bufs=1, space="PSUM") as spsum,
        ):
            # PE p-state warmup: dep-free dummy matmuls that run during the
            # DMA-init preamble so the real stream starts at full clock
            wu_rhs = singles.tile([128, 512], bf16)
            nc.vector.memset(wu_rhs[:], 0.0)
            wu_ps = spsum.tile([128, 512], f32, tag="wu")
            for wi in range(8):
                nc.tensor.matmul(
                    wu_ps[:], wu_rhs[:, 0:128], wu_rhs[:],
                    start=(wi == 0), stop=(wi == 7), skip_group_check=True,
                )
            # startup ordering: x(0,0) first (gates the whole LIF chain),
            # then just the center-tap weights (gate the first matmul),
            # then the rest interleaved
            early_x = {}
            xt = xp.tile([128, 2, L], f32, tag="x_e0")
            nc.sync.dma_start(
                out=xt[:], in_=x_d[0, 0].rearrange("(i p) l -> p i l", p=128)
            )
            early_x[(0, 0)] = xt

            w_s = singles.tile([128, K, 2, CT, 128], bf16)
            nc.sync.dma_start(out=w_s[:, 2], in_=w_d[:, 2])
            xt = xp.tile([128, 2, L], f32, tag="x_e1")
            nc.sync.dma_start(
                out=xt[:], in_=x_d[0, 1].rearrange("(i p) l -> p i l", p=128)
            )
            early_x[(0, 1)] = xt
            nc.sync.dma_start(out=w_s[:, 0:2], in_=w_d[:, 0:2])
            nc.sync.dma_start(out=w_s[:, 3:5], in_=w_d[:, 3:5])
            consts = singles.tile([128, 6, QUAD, CT], f32)
            nc.sync.dma_start(out=consts[:], in_=consts_d[:])
            ind4 = singles.tile([128, 4], bf16)
            nc.sync.dma_start(out=ind4[:], in_=ind4_d[:])
            onesb4 = singles.tile([4, 128], bf16)
            nc.sync.dma_start(out=onesb4[:], in_=onesb4_d[:])
            eps_t = singles.tile([128, 1], f32)
            nc.vector.memset(eps_t[:], EPS)

            # persistent LIF membrane state per local batch element; no
            # memset needed: the t=0 step overwrites v entirely.
            v_tiles = []
            for b in range(B_LOC):
                vt = singles.tile([128, 2, L], f32, tag=f"v{b}")
                v_tiles.append(vt)

            # tap -> (rhs_lo, rhs_hi, out_lo, out_hi) column ranges
            tap_slices = []
            for k in range(K):
                d = k - 2
                if d >= 0:
                    tap_slices.append((d, L, 0, L - d))
                else:
                    tap_slices.append((0, L + d, -d, L))
            mm_list = [(ci_t, k) for ci_t in range(2) for k in range(K)]
            mm_list.remove((0, 2))
            mm_list.insert(0, (0, 2))  # full-width center tap first (start=True)

            def emit_tail(quad, rq, ysbs, s0=0, s1=QUAD):
                """Batched groupnorm tail for samples [s0:s1] of a quad:
                per-channel bias corrections, group-sum + broadcast matmuls
                (bf16), A/B affine coefficients, then per-sample normalize +
                store."""
                n = s1 - s0
                sl = slice(s0, s1)
                # t1 = r + 512 b ; t2 = (r * 2b + q) + 512 b^2  -> bf16
                t12 = smallsb.tile([128, 2, n, CT], bf16, tag=f"t12_{n}")
                nc.vector.tensor_add(out=t12[:, 0], in0=rq[:, 0, sl], in1=consts[:, 0, sl])
                tmp = smallsb.tile([128, n, CT], f32, tag=f"tmp_{n}")
                nc.vector.tensor_mul(out=tmp[:], in0=rq[:, 0, sl], in1=consts[:, 1, sl])
                nc.vector.tensor_add(out=tmp[:], in0=tmp[:], in1=rq[:, 1, sl])
                nc.vector.tensor_add(out=t12[:, 1], in0=tmp[:], in1=consts[:, 2, sl])
                # group sums over the 32-channel blocks: [4, (stat, smp, ct)]
                # (PSUM tiles stay QUAD-wide and are sliced: banks are scarce)
                gs = spsum.tile([4, 2, QUAD, CT], f32, tag="gs")
                nc.tensor.matmul(
                    gs[0:4, :, 0:n, :], ind4[:], t12[:], start=True, stop=True
                )
                # mu = T1/N ; varN = T2 - T1*mu ; kappa = rsqrt(varN/N + eps)
                mu = smallsb.tile([4, n, CT], f32, tag=f"mu_{n}")
                nc.vector.tensor_scalar(
                    out=mu[:], in0=gs[0:4, 0, 0:n], scalar1=1.0 / NORM_N,
                    scalar2=None, op0=Alu.mult,
                )
                vn = smallsb.tile([4, n, CT], f32, tag=f"vn_{n}")
                nc.vector.tensor_mul(out=vn[:], in0=gs[0:4, 0, 0:n], in1=mu[:])
                nc.vector.tensor_sub(out=vn[:], in0=gs[0:4, 1, 0:n], in1=vn[:])
                mk = smallsb.tile([4, 2, n, CT], bf16, tag=f"mk_{n}")
                nc.vector.tensor_copy(out=mk[:, 0], in_=mu[:])
                nc.scalar.activation(
                    out=vn[:], in_=vn[:], func=Act.Sqrt,
                    bias=eps_t[0:4], scale=1.0 / NORM_N,
                )
                with nc.allow_low_precision(reason="kappa in bf16 is fine for 2e-2 budget"):
                    nc.vector.reciprocal(out=mk[:, 1], in_=vn[:])
                # broadcast mu/kappa back to 128 channels: [128, (muk, smp, ct)]
                bc = spsum.tile([128, 2, QUAD, CT], f32, tag="bc")
                nc.tensor.matmul(
                    bc[:, :, 0:n, :], onesb4[:], mk[:], start=True, stop=True
                )
                # A = kappa*gamma ; B = (b - mu)*A + beta
                ab = smallsb.tile([128, 2, n, CT], f32, tag=f"ab_{n}")
                nc.vector.tensor_mul(out=ab[:, 0], in0=bc[:, 1, 0:n], in1=consts[:, 3, sl])
                scr = smallsb.tile([128, n, CT], f32, tag=f"scr_{n}")
                nc.vector.tensor_sub(out=scr[:], in0=consts[:, 4, sl], in1=bc[:, 0, 0:n])
                nc.vector.tensor_mul(out=scr[:], in0=scr[:], in1=ab[:, 0])
                nc.vector.tensor_add(out=ab[:, 1], in0=scr[:], in1=consts[:, 5, sl])
                for si, (t, b) in enumerate(quad[s0:s1]):
                    for ct in range(CT):
                        y_sb = ysbs[s0 + si][ct]
                        # out = y*A + B: split between DVE (tensor_scalar,
                        # 2x mode) and ScalarE so neither becomes critical
                        if ct == 0:
                            nc.vector.tensor_scalar(
                                out=y_sb[:], in0=y_sb[:],
                                scalar1=ab[:, 0, si, ct : ct + 1],
                                scalar2=ab[:, 1, si, ct : ct + 1],
                                op0=Alu.mult, op1=Alu.add,
                            )
                        else:
                            nc.scalar.activation(
                                out=y_sb[:], in_=y_sb[:], func=Act.Identity,
                                bias=ab[:, 1, si, ct : ct + 1],
                                scale=ab[:, 0, si, ct : ct + 1],
                            )
                        dma_q = nc.sync if ct == 0 else nc.gpsimd
                        dma_q.dma_start(
                            out=y_d[t, b].rearrange("(i p) l -> p i l", p=128)[:, ct, :],
                            in_=y_sb[:],
                        )

            samples = [(t, b) for t in range(T) for b in range(B_LOC)]
            quad_state = {}  # quad index -> (quad, rq, ysbs)
            for i, (t, b) in enumerate(samples):
                qi, si = divmod(i, QUAD)
                if si == 0:
                    rq = smallsb.tile([128, 2, QUAD, CT], f32)  # (stat, smp, ct)
                    quad_state[qi] = ([], rq, [])
                quad, rq, ysbs = quad_state[qi]
                quad.append((t, b))
                if True:
                    xt = early_x.pop((t, b), None)
                    if xt is None:
                        xt = xp.tile([128, 2, L], f32)
                        nc.sync.dma_start(
                            out=xt[:],
                            in_=x_d[t, b].rearrange("(i p) l -> p i l", p=128),
                        )
                    v = v_tiles[b]
                    # LIF step, bit-matching the reference op order
                    if t == 0:
                        nc.vector.tensor_scalar(
                            out=v[:], in0=xt[:], scalar1=0.5, scalar2=None,
                            op0=Alu.mult,
                        )
                    else:
                        nc.vector.tensor_sub(out=xt[:], in0=xt[:], in1=v[:])
                        nc.vector.scalar_tensor_tensor(
                            out=v[:], in0=xt[:], scalar=0.5, in1=v[:],
                            op0=Alu.mult, op1=Alu.add,
                        )
                    st = sp.tile([128, 2, L], bf16)
                    nc.vector.tensor_scalar(
                        out=st[:], in0=v[:], scalar1=0.5, scalar2=None,
                        op0=Alu.is_ge,
                    )
                    if t < T - 1:
                        nc.vector.scalar_tensor_tensor(
                            out=v[:], in0=v[:], scalar=0.5, in1=v[:],
                            op0=Alu.is_lt, op1=Alu.mult,
                        )

                    # conv + stats per co-tile
                    pair = []
                    for ct in range(CT):
                        yp = ypsum.tile([128, L], f32)
                        for mi, (ci_t, k) in enumerate(mm_list):
                            rl, rh, ol, oh = tap_slices[k]
                            nc.tensor.matmul(
                                yp[:, ol:oh],
                                w_s[:, k, ci_t, ct, :],
                                st[:, ci_t, rl:rh],
                                start=(mi == 0),
                                stop=(mi == len(mm_list) - 1),
                                skip_group_check=True,
                            )
                        y_sb = ysb.tile([128, L], f32)
                        # r = sum_l y (and copy PSUM -> SBUF)
                        nc.scalar.activation(
                            out=y_sb[:], in_=yp[:], func=Act.Copy,
                            accum_out=rq[:, 0, si, ct : ct + 1],
                        )
                        # q = sum_l y^2 (squares PSUM in place; last PSUM use)
                        nc.scalar.activation(
                            out=yp[:], in_=yp[:], func=Act.Square,
                            accum_out=rq[:, 1, si, ct : ct + 1],
                        )
                        pair.append(y_sb)
                    ysbs.append(pair)

                # emit the previous quad's tail once 2 samples of the current
                # quad are in the queues: late enough that its stats are done
                # by the time DVE reaches it (no head-of-line stall), early
                # enough that only the final quad's tail lands in the drain
                if si == 1 and qi >= 1:
                    emit_tail(*quad_state.pop(qi - 1))
                # the final quad drains after the matmul stream: split its
                # tail into pairs so the first half overlaps the last convs
                last_q = len(samples) // QUAD - 1
                if qi == last_q and si == 2:
                    emit_tail(*quad_state[last_q], 0, 2)
            emit_tail(*quad_state.pop(last_q), 2, QUAD)

    nc.compile()
    return nc


def _prep_host_inputs(x, conv_w, conv_b, gamma, beta):
    x = np.asarray(x, dtype=np.float32)
    conv_w = np.asarray(conv_w, dtype=np.float32)
    conv_b = np.asarray(conv_b, dtype=np.float32)
    gamma = np.asarray(gamma, dtype=np.float32)
    beta = np.asarray(beta, dtype=np.float32)

    # lhsT tiles: [ci, k, ci_t, co_t, co], single-precision bf16
    Wt = conv_w.transpose(1, 0, 2)                      # [ci_g, co_g, k]
    W6 = Wt.reshape(2, 128, CT, 128, K)                 # [ci_t, ci, co_t, co, k]
    w_host = np.ascontiguousarray(
        W6.astype(ml_dtypes.bfloat16).transpose(1, 4, 0, 2, 3)
    )

    b = conv_b
    fields = np.stack(
        [np.float32(L) * b, np.float32(2.0) * b, np.float32(L) * b * b,
         gamma, b, beta]
    )                                                   # [6, 256]
    f6 = fields.reshape(6, CT, 128)                     # [field, ct, co]
    consts = np.zeros((128, 6, QUAD, CT), np.float32)
    for ct in range(CT):
        consts[:, :, :, ct] = f6[:, ct, :].T[:, :, None]

    ind4 = np.zeros((128, 4), ml_dtypes.bfloat16)
    for ci in range(128):
        ind4[ci, ci // GPC] = 1.0
    onesb4 = np.zeros((4, 128), ml_dtypes.bfloat16)
    for co in range(128):
        onesb4[co // GPC, co] = 1.0

    shards = []
    for i in range(N_CORES):
        shards.append(
            {
                "x": np.ascontiguousarray(x[:, i * B_LOC : (i + 1) * B_LOC]),
                "w": w_host,
                "consts": consts,
                "ind4": ind4,
                "onesb4": onesb4,
            }
        )
    return shards


def kernel(x, conv_w, conv_b, gamma, beta, _trace=False):
    from concourse.bass_utils import run_bass_kernel_spmd

    if "nc" not in _COMPILED:
        _COMPILED["nc"] = _build_program()
    nc = _COMPILED["nc"]

    in_maps = _prep_host_inputs(x, conv_w, conv_b, gamma, beta)
    res = run_bass_kernel_spmd(
        nc, in_maps, list(range(N_CORES)), trace=_trace
    )
    out = np.concatenate([r["y"] for r in res.results], axis=1)
    _COMPILED["last_result"] = res
    return out
